# revision 4
# baseline (speedup 1.0000x reference)
"""Trainium2 Bass kernel for nn_Bottleneck_11416023073044 (RFAConv bottleneck).

Data-parallel: 1 sample per NeuronCore (8 cores). cl-major interleave:
partition = cl*9 + idx (idx = patch index i for inputs / n for outputs)
within each 14-channel group (last group has 2).

Per-core pipeline per chunk (CS=400 = 5 rows x 80):
  cv1:    hp = silu(a1*(W1 @ x) + c1)   ACT sigmoid + DVE affine + DVE mul,
          interleaved with x piece loads and hp9d quarter writes
  strips: hp -> 9 shifted DRAM copies (quarter-frame windows) ->
          [126,10,820] read-sc tiles (10 rows, A/B buffers)
  z:      pz = zb_g^T @ strips_g                (bf16 matmul)
  e:      e9all[:,g,:] = exp(pz + cg9)          ACT -> bf16
  D:      pd[32m:+32, bank] = ones32^T @ e9     4 groups per PSUM bank at
                                                32-aligned tile_position
  recip:  rd = 1/pd                             one batched DVE reciprocal
  rep:    repall[:,g,:] <- rd slice             replicating DMA (stride-0
                                                inner dim), split SP/SWDGE
  q:      q1all = strips*e9all; q2all = q1all*repall   one DVE tt op each
          over all 10 groups ([126, 4000] per op, 2x_1p mode)
  out:    po[h] += wc9_g^T @ q2all[:,g,:]       bf16 matmuls, 10 groups
  final:  out = x + relu(a2*po + c2)            ACT/DVE affines + Pool adds
Software-pipelined two chunks deep; chunks 0-1 are warmed during phase A.
"""
import numpy as np
import ml_dtypes

EPS = 1e-5
B, C1, C2, H, W = 8, 256, 256, 80, 80
C_ = C2 // 2          # 128
NG = 10               # channel groups
GC = 14               # channels per group (last group has 2)
HP = H + 2            # 82
S = H * W             # 6400
CH_ROWS = 5           # rows per compute chunk
CS = CH_ROWS * W      # 400 columns per chunk
NCH = H // CH_ROWS    # 16 chunks
RSC_ROWS = 10         # rows per strip read-superchunk
NRSC = H // RSC_ROWS  # 8
RSUBS = RSC_ROWS // CH_ROWS  # 4 chunks per read-sc
OSC_ROWS = 20         # rows per output store block
FW = H * HP           # 6560 flat window length per shifted copy
HPFLAT = HP * HP + 2  # 6726


def _grp(g):
    c0 = g * GC
    ncg = min(GC, C_ - c0)
    return c0, ncg, 9 * ncg


def _fold_constants(W1, g1, b1, m1, v1, Wg, bg, gg, bgw, mg, vg, Wc, bc, g2, b2,
                    m2, v2):
    """Fold BN affines; build cl-major interleaved stationaries."""
    f32 = np.float32
    bf16 = ml_dtypes.bfloat16
    cst = {}
    a1 = (g1 / np.sqrt(v1 + EPS)).astype(f32)
    c1 = (b1 - m1 * a1).astype(f32)
    cst['a1c1'] = np.stack([a1, c1], axis=1)                  # [128, 2] f32

    cst['w1t'] = np.ascontiguousarray(W1.T).astype(f32)       # [256, 128] f32

    ag = gg / np.sqrt(vg + EPS)                               # [128, 9]
    A = (ag[:, :, None] * Wg).astype(f32)                     # [c, n, i]
    cg = (ag * (bg - mg) + bgw).astype(f32)                   # [128, 9]

    zb = np.zeros((126, NG, 126), f32)
    ones32 = np.zeros((126, 2, 32), f32)   # variant 0: ncg=14, 1: ncg=2
    cg9 = np.zeros((126, NG), f32)
    wc9 = np.zeros((126, NG, C2), f32)
    for g in range(NG):
        c0, ncg, P = _grp(g)
        for cl in range(ncg):
            c = c0 + cl
            for n in range(9):
                m = cl * 9 + n
                cg9[m, g] = cg[c, n]
                wc9[m, g, :] = Wc[:, c, n]
                for i in range(9):
                    zb[cl * 9 + i, g, m] = A[c, n, i]
    for v, ncg in ((0, GC), (1, 2)):
        for cl in range(ncg):
            for n in range(9):
                for j in range(32):
                    if j % ncg == cl:
                        ones32[cl * 9 + n, v, j] = 1.0
    cst['zb'] = zb.astype(bf16)
    cst['ones32'] = ones32.astype(bf16)
    cst['cg9'] = cg9
    cst['wc9'] = wc9.astype(bf16)

    a2 = (g2 / np.sqrt(v2 + EPS)).astype(f32)
    c2 = (b2 + a2 * (bc - m2)).astype(f32)
    a2c2 = np.zeros((C_, 2, 2), f32)
    for h in range(2):
        a2c2[:, h, 0] = a2[h * C_:(h + 1) * C_]
        a2c2[:, h, 1] = c2[h * C_:(h + 1) * C_]
    cst['a2c2'] = a2c2
    return cst


_PROGRAM = None


def _build_program():
    import concourse.bass as bass
    import concourse.tile as tile
    from concourse import mybir

    dt = mybir.dt
    AF = mybir.ActivationFunctionType
    ALU = mybir.AluOpType

    nc = bass.Bass("TRN2", target_bir_lowering=False, debug=False)

    xs_d = nc.dram_tensor("xs", [C1, S], dt.float32r, kind="ExternalInput")
    w1t_d = nc.dram_tensor("w1t", [C1, C_], dt.float32r, kind="ExternalInput")
    a1c1_d = nc.dram_tensor("a1c1", [C_, 2], dt.float32, kind="ExternalInput")
    zb_d = nc.dram_tensor("zb", [126, NG, 126], dt.bfloat16, kind="ExternalInput")
    ones_d = nc.dram_tensor("ones32", [126, 2, 32], dt.bfloat16, kind="ExternalInput")
    cg9_d = nc.dram_tensor("cg9", [126, NG], dt.float32, kind="ExternalInput")
    wc9_d = nc.dram_tensor("wc9", [126, NG, C2], dt.bfloat16, kind="ExternalInput")
    a2c2_d = nc.dram_tensor("a2c2", [C_, 2, 2], dt.float32, kind="ExternalInput")
    out_d = nc.dram_tensor("out", [C2, S], dt.float32, kind="ExternalOutput")
    hp9d = nc.dram_tensor("hp9d", [9, C_, FW], dt.bfloat16)

    with tile.TileContext(nc) as tc:
        with tc.tile_pool(name="singles", bufs=1) as singles, \
             tc.tile_pool(name="strips", bufs=1) as strips, \
             tc.tile_pool(name="q1p", bufs=3) as q1p, \
             tc.tile_pool(name="q2p", bufs=1) as q2p, \
             tc.tile_pool(name="rdp", bufs=2) as rdp, \
             tc.tile_pool(name="actp", bufs=2) as actp, \
             tc.tile_pool(name="obp", bufs=1) as obp, \
             tc.tile_pool(name="psz", bufs=3, space="PSUM") as psz, \
             tc.tile_pool(name="psd", bufs=1, space="PSUM") as psd, \
             tc.tile_pool(name="pso", bufs=1, space="PSUM") as pso:

            # ---- resident tiles + constant loads ----
            x2 = [singles.tile([C_, S], dt.float32r, tag=f"x{k}", name=f"x{k}")
                  for k in range(2)]
            w1t = [singles.tile([C_, C_], dt.float32r, tag=f"w1t{k}", name=f"w1t{k}")
                   for k in range(2)]
            for k in range(2):
                nc.sync.dma_start(out=w1t[k][:], in_=w1t_d[k * C_:(k + 1) * C_, :])
            a1c1 = singles.tile([C_, 2], dt.float32, tag="a1c1", name="a1c1")
            nc.sync.dma_start(out=a1c1[:], in_=a1c1_d[:])
            def emit_x_piece(piece):
                sl = slice(piece * (S // 4), (piece + 1) * (S // 4))
                for k in range(2):
                    nc.sync.dma_start(out=x2[k][:, sl],
                                      in_=xs_d[k * C_:(k + 1) * C_, sl])
            emit_x_piece(0)
            zb = singles.tile([126, NG, 126], dt.bfloat16, tag="zb", name="zb")
            nc.sync.dma_start(out=zb[:], in_=zb_d[:])
            ones32 = singles.tile([126, 2, 32], dt.bfloat16, tag="ones32", name="ones32")
            nc.sync.dma_start(out=ones32[:], in_=ones_d[:])
            cg9 = singles.tile([126, NG], dt.float32, tag="cg9", name="cg9")
            nc.sync.dma_start(out=cg9[:], in_=cg9_d[:])
            wc9 = singles.tile([126, NG, C2], dt.bfloat16, tag="wc9", name="wc9")
            nc.sync.dma_start(out=wc9[:], in_=wc9_d[:])
            a2c2 = singles.tile([C_, 2, 2], dt.float32, tag="a2c2", name="a2c2")
            nc.sync.dma_start(out=a2c2[:], in_=a2c2_d[:])

            hpfl = singles.tile([C_, HPFLAT], dt.bfloat16, tag="hp", name="hp")
            # zero only the pad border: top row, bottom row (+2 tail), side cols
            nc.vector.memset(hpfl[:, 0:HP], 0.0)
            nc.vector.memset(hpfl[:, (HP - 1) * HP:HPFLAT], 0.0)
            sideap = bass.AP(tensor=hpfl[:].tensor, offset=hpfl[:].offset + HP,
                             ap=[[HPFLAT, C_], [HP, H], [HP - 1, 2]])
            nc.vector.memset(sideap, 0.0)

            # big per-chunk group-batched tiles (explicit A/B by chunk parity)
            NPAR = 3
            e9all = [singles.tile([126, NG, CS], dt.bfloat16, tag=f"e9all{p}",
                                  name=f"e9all{p}") for p in range(NPAR)]
            repall = [singles.tile([126, NG, CS], dt.bfloat16, tag=f"repall{p}",
                                   name=f"repall{p}") for p in range(NPAR)]
            # one-time zero of group-9 stale partitions (never written later)
            for p in range(NPAR):
                nc.vector.memset(e9all[p][:, NG - 1, :], 0.0)
                nc.vector.memset(repall[p][:, NG - 1, :], 0.0)

            hp3 = hpfl[:, 0:HP * HP].rearrange("p (a b) -> p a b", a=HP)

            # ---- phase A: cv1 into padded frame ----
            def emit_cv1(ch):
                y0 = ch * CH_ROWS
                ph = psz.tile([C_, CS], dt.float32, tag="pz", name="ph")
                for k in range(2):
                    nc.tensor.matmul(
                        out=ph[:], lhsT=w1t[k][:],
                        rhs=x2[k][:, y0 * W:(y0 + CH_ROWS) * W],
                        start=(k == 0), stop=(k == 1))
                yb = actp.tile([C_, CS], dt.bfloat16, tag="yb", name="yb")
                nc.vector.tensor_scalar(
                    out=yb[:], in0=ph[:], scalar1=a1c1[:, 0:1],
                    scalar2=a1c1[:, 1:2], op0=ALU.mult, op1=ALU.add)
                sg = actp.tile([C_, CS], dt.bfloat16, tag="sg", name="sg")
                nc.scalar.activation(out=sg[:], in_=ph[:], func=AF.Sigmoid,
                                     scale=a1c1[:, 0:1], bias=a1c1[:, 1:2])
                nc.vector.tensor_mul(hp3[:, 1 + y0:1 + y0 + CH_ROWS, 1:1 + W],
                                     yb[:], sg[:])

            def emit_hp9d_writes(q):
                f0 = q * (FW // 4)
                for i in range(9):
                    d = (i // 3) * HP + (i % 3)
                    nc.sync.dma_start(
                        out=hp9d[i, :, f0:f0 + FW // 4],
                        in_=hpfl[:, d + f0:d + f0 + FW // 4])

            # ---- persistent PSUM D banks (one 3-bank tile) ----
            pd = psd.tile([C_, 3, 512], dt.float32, tag="pd", name="pd")
            nc.vector.memset(pd[:], 1.0)

            stp = [strips.tile([126, NG, RSC_ROWS * HP], dt.bfloat16,
                               tag=f"st{p}", name=f"st{p}") for p in range(2)]
            for p in range(2):
                nc.vector.memset(stp[p][:, NG - 1, :], 0.0)
            obt = [obp.tile([C_, OSC_ROWS * W], dt.float32, tag=f"ob{h}",
                            name=f"ob{h}") for h in range(2)]

            def emit_strip_reads(rsc):
                ys = rsc * RSC_ROWS
                st = stp[rsc % 2]
                eng = nc.gpsimd if rsc == 1 else nc.sync
                for g in range(NG):
                    c0, ncg, P = _grp(g)
                    srcap = bass.AP(
                        tensor=hp9d[0].tensor, offset=c0 * FW + ys * HP,
                        ap=[[FW, ncg], [C_ * FW, 9], [1, RSC_ROWS * HP]])
                    eng.dma_start(out=st[0:P, g, :], in_=srcap)
                return st

            def emit_d(g, e9a):
                c0, ncg, P = _grp(g)
                v = 0 if ncg == GC else 1
                b, m = g // 4, g % 4
                nc.tensor.matmul(out=pd[32 * m:32 * m + 32, b, 0:CS],
                                 lhsT=ones32[0:P, v, :], rhs=e9a[0:P, g, :],
                                 start=True, stop=True, tile_position=(0, 32 * m),
                                 skip_group_check=True)

            def emit_recip_reps(n, rd, repa):
                with nc.allow_low_precision(reason="softmax recip bf16"):
                    nc.vector.reciprocal(
                        out=rd[:].rearrange("p (b e) -> p b e", b=3),
                        in_=pd[:, :, 0:CS])
                for g in range(NG):
                    c0, ncg, P = _grp(g)
                    b, m = g // 4, g % 4
                    srcap = bass.AP(
                        tensor=rd[:].tensor,
                        offset=rd[:].offset + (32 * m) * (3 * CS) + b * CS,
                        ap=[[3 * CS, ncg], [0, 9], [1, CS]])
                    eng = nc.gpsimd if g % 2 == 0 else nc.sync
                    eng.dma_start(out=repa[0:P, g, :], in_=srcap)

            def emit_zde(n, st):
                """z, exp, D, recip, rep, q1 for chunk n."""
                sub = n % RSUBS
                par = n % NPAR
                e9a, repa = e9all[par], repall[par]
                str3 = st[:].rearrange("p g (r c) -> p g r c", c=HP)
                rd = rdp.tile([C_, 3 * CS], dt.bfloat16, tag="rd", name=f"rd{n}")

                for g in range(NG):
                    c0, ncg, P = _grp(g)
                    rhs = str3[0:P, g, sub * CH_ROWS:(sub + 1) * CH_ROWS, 0:W]
                    pz = psz.tile([126, CS], dt.float32, tag="pz",
                                  name=f"pz{n}_{g}")
                    nc.tensor.matmul(out=pz[0:P, :], lhsT=zb[0:P, g, :][:, 0:P],
                                     rhs=rhs, start=True, stop=True)
                    nc.scalar.activation(out=e9a[0:P, g, :], in_=pz[0:P, :],
                                         func=AF.Exp, bias=cg9[0:P, g:g + 1])
                    if g > 0:
                        emit_d(g - 1, e9a)
                emit_d(NG - 1, e9a)
                emit_recip_reps(n, rd, repa)
                # one batched q1 over all groups
                q1 = q1p.tile([126, NG, CS], dt.bfloat16, tag="q1", name=f"q1{n}")
                sview = str3[0:126, :, sub * CH_ROWS:(sub + 1) * CH_ROWS, 0:W]
                nc.vector.tensor_mul(q1[:], sview, e9a[:])
                return q1

            def emit_out(n, q1):
                """q2 mul + out matmuls + finals for chunk n."""
                par = n % NPAR
                repa = repall[par]
                po = [pso.tile([C_, CS], dt.float32, tag=f"po{h}",
                               name=f"po{h}_{n}") for h in range(2)]
                q2 = q2p.tile([126, NG, CS], dt.bfloat16, tag="q2", name=f"q2{n}")
                nc.vector.tensor_mul(q2[:], q1[:], repa[:])
                for g in range(NG):
                    c0, ncg, P = _grp(g)
                    for h in range(2):
                        nc.tensor.matmul(
                            out=po[h][:], lhsT=wc9[0:P, g, h * C_:(h + 1) * C_],
                            rhs=q2[0:P, g, :], start=(g == 0), stop=(g == NG - 1))
                # finals: affines on ACT/DVE, residual adds on Pool
                osub = n % (OSC_ROWS // CH_ROWS)
                osl = slice(osub * CS, (osub + 1) * CS)
                xsl = slice(n * CS, (n + 1) * CS)
                ob = obt
                t0 = actp.tile([C_, CS], dt.bfloat16, tag="t0", name=f"t0_{n}")
                nc.scalar.activation(out=t0[:], in_=po[0][:], func=AF.Relu,
                                     scale=a2c2[:, 0, 0:1], bias=a2c2[:, 0, 1:2])
                nc.gpsimd.tensor_add(ob[0][:, osl], t0[:],
                                     x2[0][:, xsl].bitcast(dt.float32))
                t1 = actp.tile([C_, CS], dt.bfloat16, tag="t1", name=f"t1_{n}")
                with nc.allow_low_precision(reason="bn affine bf16 ok"):
                    nc.vector.tensor_scalar(
                        out=t1[:], in0=po[1][:], scalar1=a2c2[:, 1, 0:1],
                        scalar2=a2c2[:, 1, 1:2], op0=ALU.mult, op1=ALU.add)
                t1r = actp.tile([C_, CS], dt.bfloat16, tag="t1r", name=f"t1r_{n}")
                nc.vector.tensor_scalar(
                    out=t1r[:], in0=t1[:], scalar1=0.0, scalar2=None,
                    op0=ALU.max)
                nc.gpsimd.tensor_add(ob[1][:, osl], t1r[:],
                                     x2[1][:, xsl].bitcast(dt.float32))
                return ob

            def emit_store(n, ob):
                osc = n // (OSC_ROWS // CH_ROWS)
                for h in range(2):
                    nc.sync.dma_start(
                        out=out_d[h * C_:(h + 1) * C_,
                                  osc * OSC_ROWS * W:(osc + 1) * OSC_ROWS * W],
                        in_=ob[h][:])

            # ---- phase A interleaved with strip production ----
            # quarter q covers shifted-frame rows 20q..20q+19 (src rows up to
            # 20q+21 -> cv1 chunks through ceil((20q+21-1)/5)=4q+4)
            sts = {}
            warm = []
            for ch in range(NCH):
                emit_cv1(ch)
                if ch % 4 == 0 and ch // 4 < 3:
                    emit_x_piece(ch // 4 + 1)
                if ch == 4:
                    emit_hp9d_writes(0)
                    sts[0] = emit_strip_reads(0)
                    sts[1] = emit_strip_reads(1)
                elif ch == 9:
                    emit_hp9d_writes(1)
                elif ch == 11:
                    warm.append((0, emit_zde(0, sts[0])))
                elif ch == 13:
                    warm.append((1, emit_zde(1, sts[0])))
                elif ch == 14:
                    emit_hp9d_writes(2)
                elif ch == 15:
                    emit_hp9d_writes(3)

            # ---- main loop, software-pipelined by one chunk ----
            OS_SUBS = OSC_ROWS // CH_ROWS
            from collections import deque
            pend = deque(warm)
            DEPTH = 2

            def drain_one():
                pn, pq1 = pend.popleft()
                ob = emit_out(pn, pq1)
                if pn % OS_SUBS == OS_SUBS - 1:
                    emit_store(pn, ob)

            for rsc in range(NRSC):
                st = sts.pop(rsc) if rsc in sts else emit_strip_reads(rsc)
                for sub in range(RSUBS):
                    n = rsc * RSUBS + sub
                    if n < len(warm):
                        continue
                    q1 = emit_zde(n, st)
                    pend.append((n, q1))
                    if len(pend) > DEPTH:
                        drain_one()
            while pend:
                drain_one()

    _split_excess_waits(nc)
    return nc


def _split_excess_waits(nc):
    """This walrus build rejects >1 sync-wait on TPB_CTRL instructions and
    >2 elsewhere; redistribute onto same-engine wait-nops inserted before."""
    import concourse.mybir as mybir
    cnt = [0]
    for bb in nc.main_func.blocks:
        new_list = []
        changed = False
        for ins in bb.instructions:
            si = ins.sync_info
            lim = 1
            if si is not None and si.on_wait is not None and len(si.on_wait) > lim:
                waits = list(si.on_wait)
                head, tail = waits[:-lim], waits[-lim:]
                for w in head:
                    nop = mybir.InstNoOp(name=f"waitsplit-{cnt[0]}", ins=[], outs=[])
                    cnt[0] += 1
                    nop.engine = ins.engine
                    nop.sync_info = mybir.SyncInfo(on_wait=[w], on_update=[])
                    nop.bass_nofuse = True
                    try:
                        nc.register_instruction(nop)
                    except Exception:
                        pass
                    new_list.append(nop)
                ins.sync_info = mybir.SyncInfo(
                    on_wait=tail, on_update=list(si.on_update or []))
                changed = True
            new_list.append(ins)
        if changed:
            bb.instructions[:] = new_list


def _get_program():
    global _PROGRAM
    if _PROGRAM is None:
        _PROGRAM = _build_program()
    return _PROGRAM


def kernel(**inputs):
    from concourse.bass_utils import run_bass_kernel_spmd

    x = np.asarray(inputs['x'], dtype=np.float32)
    cst = _fold_constants(**{k: np.asarray(v, dtype=np.float32)
                             for k, v in inputs.items() if k != 'x'})
    nc = _get_program()
    base = {
        'w1t': cst['w1t'], 'a1c1': cst['a1c1'], 'zb': cst['zb'],
        'ones32': cst['ones32'], 'cg9': cst['cg9'],
        'wc9': cst['wc9'], 'a2c2': cst['a2c2'],
    }
    in_maps = [dict(base, xs=np.ascontiguousarray(x[b].reshape(C1, S)))
               for b in range(B)]
    res = run_bass_kernel_spmd(nc, in_maps, list(range(B)))
    out = np.stack([res.results[b]['out'].reshape(C2, H, W) for b in range(B)])
    return out.astype(np.float32)


# revision 5
# speedup vs baseline: 1.0172x; 1.0172x over previous
"""Trainium2 Bass kernel for nn_Bottleneck_11416023073044 (RFAConv bottleneck).

Data-parallel: 1 sample per NeuronCore (8 cores). cl-major interleave:
partition = cl*9 + idx (idx = patch index i for inputs / n for outputs)
within each 14-channel group (last group has 2).

Per-core pipeline per chunk (CS=400 = 5 rows x 80):
  cv1:    hp = silu(a1*(W1 @ x) + c1)   ACT sigmoid + DVE affine + DVE mul,
          interleaved with x piece loads and hp9d quarter writes
  strips: hp -> 9 shifted DRAM copies (quarter-frame windows) ->
          [126,10,820] read-sc tiles (10 rows, A/B buffers)
  z:      pz = zb_g^T @ strips_g                (bf16 matmul)
  e:      e9all[:,g,:] = exp(pz + cg9)          ACT -> bf16
  D:      pd[32m:+32, bank] = ones32^T @ e9     4 groups per PSUM bank at
                                                32-aligned tile_position
  recip:  rd = 1/pd                             one batched DVE reciprocal
  rep:    repall[:,g,:] <- rd slice             replicating DMA (stride-0
                                                inner dim), split SP/SWDGE
  q:      q1all = strips*e9all; q2all = q1all*repall   one DVE tt op each
          over all 10 groups ([126, 4000] per op, 2x_1p mode)
  out:    po[h] += wc9_g^T @ q2all[:,g,:]       bf16 matmuls, 10 groups
  final:  out = x + relu(a2*po + c2)            ACT/DVE affines + Pool adds
Software-pipelined two chunks deep; chunks 0-1 are warmed during phase A.
"""
import numpy as np
import ml_dtypes

EPS = 1e-5
B, C1, C2, H, W = 8, 256, 256, 80, 80
C_ = C2 // 2          # 128
NG = 10               # channel groups
GC = 14               # channels per group (last group has 2)
HP = H + 2            # 82
S = H * W             # 6400
CH_ROWS = 5           # rows per compute chunk
CS = CH_ROWS * W      # 400 columns per chunk
NCH = H // CH_ROWS    # 16 chunks
RSC_ROWS = 10         # rows per strip read-superchunk
NRSC = H // RSC_ROWS  # 8
RSUBS = RSC_ROWS // CH_ROWS  # 4 chunks per read-sc
OSC_ROWS = 20         # rows per output store block
FW = H * HP           # 6560 flat window length per shifted copy
HPFLAT = HP * HP + 2  # 6726


def _grp(g):
    c0 = g * GC
    ncg = min(GC, C_ - c0)
    return c0, ncg, 9 * ncg


def _fold_constants(W1, g1, b1, m1, v1, Wg, bg, gg, bgw, mg, vg, Wc, bc, g2, b2,
                    m2, v2):
    """Fold BN affines; build cl-major interleaved stationaries."""
    f32 = np.float32
    bf16 = ml_dtypes.bfloat16
    cst = {}
    a1 = (g1 / np.sqrt(v1 + EPS)).astype(f32)
    c1 = (b1 - m1 * a1).astype(f32)
    cst['a1c1'] = np.stack([a1, c1], axis=1)                  # [128, 2] f32

    cst['w1t'] = np.ascontiguousarray(W1.T).astype(f32)       # [256, 128] f32

    ag = gg / np.sqrt(vg + EPS)                               # [128, 9]
    A = (ag[:, :, None] * Wg).astype(f32)                     # [c, n, i]
    cg = (ag * (bg - mg) + bgw).astype(f32)                   # [128, 9]

    zb = np.zeros((126, NG, 126), f32)
    ones32 = np.zeros((126, 2, 32), f32)   # variant 0: ncg=14, 1: ncg=2
    cg9 = np.zeros((126, NG), f32)
    wc9 = np.zeros((126, NG, C2), f32)
    for g in range(NG):
        c0, ncg, P = _grp(g)
        for cl in range(ncg):
            c = c0 + cl
            for n in range(9):
                m = cl * 9 + n
                cg9[m, g] = cg[c, n]
                wc9[m, g, :] = Wc[:, c, n]
                for i in range(9):
                    zb[cl * 9 + i, g, m] = A[c, n, i]
    for v, ncg in ((0, GC), (1, 2)):
        for cl in range(ncg):
            for n in range(9):
                for j in range(32):
                    if j % ncg == cl:
                        ones32[cl * 9 + n, v, j] = 1.0
    cst['zb'] = zb.astype(bf16)
    cst['ones32'] = ones32.astype(bf16)
    cst['cg9'] = cg9
    cst['wc9'] = wc9.astype(bf16)

    a2 = (g2 / np.sqrt(v2 + EPS)).astype(f32)
    c2 = (b2 + a2 * (bc - m2)).astype(f32)
    a2c2 = np.zeros((C_, 2, 2), f32)
    for h in range(2):
        a2c2[:, h, 0] = a2[h * C_:(h + 1) * C_]
        a2c2[:, h, 1] = c2[h * C_:(h + 1) * C_]
    cst['a2c2'] = a2c2
    return cst


_PROGRAM = None


def _build_program():
    import concourse.bass as bass
    import concourse.tile as tile
    from concourse import mybir

    dt = mybir.dt
    AF = mybir.ActivationFunctionType
    ALU = mybir.AluOpType

    nc = bass.Bass("TRN2", target_bir_lowering=False, debug=False)

    xs_d = nc.dram_tensor("xs", [C1, S], dt.float32r, kind="ExternalInput")
    w1t_d = nc.dram_tensor("w1t", [C1, C_], dt.float32r, kind="ExternalInput")
    a1c1_d = nc.dram_tensor("a1c1", [C_, 2], dt.float32, kind="ExternalInput")
    zb_d = nc.dram_tensor("zb", [126, NG, 126], dt.bfloat16, kind="ExternalInput")
    ones_d = nc.dram_tensor("ones32", [126, 2, 32], dt.bfloat16, kind="ExternalInput")
    cg9_d = nc.dram_tensor("cg9", [126, NG], dt.float32, kind="ExternalInput")
    wc9_d = nc.dram_tensor("wc9", [126, NG, C2], dt.bfloat16, kind="ExternalInput")
    a2c2_d = nc.dram_tensor("a2c2", [C_, 2, 2], dt.float32, kind="ExternalInput")
    out_d = nc.dram_tensor("out", [C2, S], dt.float32, kind="ExternalOutput")
    hp9d = nc.dram_tensor("hp9d", [9, C_, FW], dt.bfloat16)

    with tile.TileContext(nc) as tc:
        with tc.tile_pool(name="singles", bufs=1) as singles, \
             tc.tile_pool(name="strips", bufs=1) as strips, \
             tc.tile_pool(name="q1p", bufs=3) as q1p, \
             tc.tile_pool(name="q2p", bufs=1) as q2p, \
             tc.tile_pool(name="rdp", bufs=2) as rdp, \
             tc.tile_pool(name="actp", bufs=2) as actp, \
             tc.tile_pool(name="obp", bufs=1) as obp, \
             tc.tile_pool(name="psz", bufs=3, space="PSUM") as psz, \
             tc.tile_pool(name="psd", bufs=1, space="PSUM") as psd, \
             tc.tile_pool(name="pso", bufs=1, space="PSUM") as pso:

            # ---- resident tiles + constant loads ----
            x2 = [singles.tile([C_, S], dt.float32r, tag=f"x{k}", name=f"x{k}")
                  for k in range(2)]
            w1t = [singles.tile([C_, C_], dt.float32r, tag=f"w1t{k}", name=f"w1t{k}")
                   for k in range(2)]
            for k in range(2):
                nc.sync.dma_start(out=w1t[k][:], in_=w1t_d[k * C_:(k + 1) * C_, :])
            a1c1 = singles.tile([C_, 2], dt.float32, tag="a1c1", name="a1c1")
            nc.sync.dma_start(out=a1c1[:], in_=a1c1_d[:])
            def emit_x_piece(piece):
                sl = slice(piece * (S // 4), (piece + 1) * (S // 4))
                for k in range(2):
                    nc.sync.dma_start(out=x2[k][:, sl],
                                      in_=xs_d[k * C_:(k + 1) * C_, sl])
            emit_x_piece(0)
            zb = singles.tile([126, NG, 126], dt.bfloat16, tag="zb", name="zb")
            nc.sync.dma_start(out=zb[:], in_=zb_d[:])
            ones32 = singles.tile([126, 2, 32], dt.bfloat16, tag="ones32", name="ones32")
            nc.sync.dma_start(out=ones32[:], in_=ones_d[:])
            cg9 = singles.tile([126, NG], dt.float32, tag="cg9", name="cg9")
            nc.sync.dma_start(out=cg9[:], in_=cg9_d[:])
            wc9 = singles.tile([126, NG, C2], dt.bfloat16, tag="wc9", name="wc9")
            nc.sync.dma_start(out=wc9[:], in_=wc9_d[:])
            a2c2 = singles.tile([C_, 2, 2], dt.float32, tag="a2c2", name="a2c2")
            nc.sync.dma_start(out=a2c2[:], in_=a2c2_d[:])

            hpfl = singles.tile([C_, HPFLAT], dt.bfloat16, tag="hp", name="hp")
            # zero only the pad border: top row, bottom row (+2 tail), side cols
            nc.vector.memset(hpfl[:, 0:HP], 0.0)
            nc.vector.memset(hpfl[:, (HP - 1) * HP:HPFLAT], 0.0)
            sideap = bass.AP(tensor=hpfl[:].tensor, offset=hpfl[:].offset + HP,
                             ap=[[HPFLAT, C_], [HP, H], [HP - 1, 2]])
            nc.vector.memset(sideap, 0.0)

            # big per-chunk group-batched tiles (explicit A/B by chunk parity)
            NPAR = 3
            e9all = [singles.tile([126, NG, CS], dt.bfloat16, tag=f"e9all{p}",
                                  name=f"e9all{p}") for p in range(NPAR)]
            repall = [singles.tile([126, NG, CS], dt.bfloat16, tag=f"repall{p}",
                                   name=f"repall{p}") for p in range(NPAR)]
            # one-time zero of group-9 stale partitions (never written later)
            for p in range(NPAR):
                nc.vector.memset(e9all[p][:, NG - 1, :], 0.0)
                nc.vector.memset(repall[p][:, NG - 1, :], 0.0)

            hp3 = hpfl[:, 0:HP * HP].rearrange("p (a b) -> p a b", a=HP)

            # ---- phase A: cv1 into padded frame ----
            def emit_cv1(ch):
                y0 = ch * CH_ROWS
                ph = psz.tile([C_, CS], dt.float32, tag="pz", name="ph")
                for k in range(2):
                    nc.tensor.matmul(
                        out=ph[:], lhsT=w1t[k][:],
                        rhs=x2[k][:, y0 * W:(y0 + CH_ROWS) * W],
                        start=(k == 0), stop=(k == 1))
                yb = actp.tile([C_, CS], dt.bfloat16, tag="yb", name="yb")
                nc.vector.tensor_scalar(
                    out=yb[:], in0=ph[:], scalar1=a1c1[:, 0:1],
                    scalar2=a1c1[:, 1:2], op0=ALU.mult, op1=ALU.add)
                sg = actp.tile([C_, CS], dt.bfloat16, tag="sg", name="sg")
                nc.scalar.activation(out=sg[:], in_=ph[:], func=AF.Sigmoid,
                                     scale=a1c1[:, 0:1], bias=a1c1[:, 1:2])
                nc.vector.tensor_mul(hp3[:, 1 + y0:1 + y0 + CH_ROWS, 1:1 + W],
                                     yb[:], sg[:])

            def emit_hp9d_writes(q):
                f0 = q * (FW // 4)
                for i in range(9):
                    d = (i // 3) * HP + (i % 3)
                    nc.sync.dma_start(
                        out=hp9d[i, :, f0:f0 + FW // 4],
                        in_=hpfl[:, d + f0:d + f0 + FW // 4])

            # ---- persistent PSUM D banks (one 3-bank tile) ----
            pd = psd.tile([C_, 3, 512], dt.float32, tag="pd", name="pd")
            nc.vector.memset(pd[:], 1.0)

            stp = [strips.tile([126, NG, RSC_ROWS * HP], dt.bfloat16,
                               tag=f"st{p}", name=f"st{p}") for p in range(2)]
            for p in range(2):
                nc.vector.memset(stp[p][:, NG - 1, :], 0.0)
            obt = [obp.tile([C_, OSC_ROWS * W], dt.float32, tag=f"ob{h}",
                            name=f"ob{h}") for h in range(2)]

            def emit_strip_reads(rsc):
                ys = rsc * RSC_ROWS
                st = stp[rsc % 2]
                for g in range(NG):
                    c0, ncg, P = _grp(g)
                    srcap = bass.AP(
                        tensor=hp9d[0].tensor, offset=c0 * FW + ys * HP,
                        ap=[[FW, ncg], [C_ * FW, 9], [1, RSC_ROWS * HP]])
                    eng = nc.gpsimd if g % 2 == 0 else nc.sync
                    eng.dma_start(out=st[0:P, g, :], in_=srcap)
                return st

            def emit_d(g, e9a):
                c0, ncg, P = _grp(g)
                v = 0 if ncg == GC else 1
                b, m = g // 4, g % 4
                nc.tensor.matmul(out=pd[32 * m:32 * m + 32, b, 0:CS],
                                 lhsT=ones32[0:P, v, :], rhs=e9a[0:P, g, :],
                                 start=True, stop=True, tile_position=(0, 32 * m),
                                 skip_group_check=True)

            def emit_recip_reps(n, rd, repa):
                with nc.allow_low_precision(reason="softmax recip bf16"):
                    nc.vector.reciprocal(
                        out=rd[:].rearrange("p (b e) -> p b e", b=3),
                        in_=pd[:, :, 0:CS])
                for g in range(NG):
                    c0, ncg, P = _grp(g)
                    b, m = g // 4, g % 4
                    srcap = bass.AP(
                        tensor=rd[:].tensor,
                        offset=rd[:].offset + (32 * m) * (3 * CS) + b * CS,
                        ap=[[3 * CS, ncg], [0, 9], [1, CS]])
                    eng = nc.gpsimd if g % 2 == 0 else nc.sync
                    eng.dma_start(out=repa[0:P, g, :], in_=srcap)

            def emit_zde(n, st):
                """z, exp, D, recip, rep, q1 for chunk n."""
                sub = n % RSUBS
                par = n % NPAR
                e9a, repa = e9all[par], repall[par]
                str3 = st[:].rearrange("p g (r c) -> p g r c", c=HP)
                rd = rdp.tile([C_, 3 * CS], dt.bfloat16, tag="rd", name=f"rd{n}")

                for g in range(NG):
                    c0, ncg, P = _grp(g)
                    rhs = str3[0:P, g, sub * CH_ROWS:(sub + 1) * CH_ROWS, 0:W]
                    pz = psz.tile([126, CS], dt.float32, tag="pz",
                                  name=f"pz{n}_{g}")
                    nc.tensor.matmul(out=pz[0:P, :], lhsT=zb[0:P, g, :][:, 0:P],
                                     rhs=rhs, start=True, stop=True)
                    nc.scalar.activation(out=e9a[0:P, g, :], in_=pz[0:P, :],
                                         func=AF.Exp, bias=cg9[0:P, g:g + 1])
                    if g > 0:
                        emit_d(g - 1, e9a)
                emit_d(NG - 1, e9a)
                emit_recip_reps(n, rd, repa)
                # one batched q1 over all groups
                q1 = q1p.tile([126, NG, CS], dt.bfloat16, tag="q1", name=f"q1{n}")
                sview = str3[0:126, :, sub * CH_ROWS:(sub + 1) * CH_ROWS, 0:W]
                nc.vector.tensor_mul(q1[:], sview, e9a[:])
                return q1

            def emit_out(n, q1):
                """q2 mul + out matmuls + finals for chunk n."""
                par = n % NPAR
                repa = repall[par]
                po = [pso.tile([C_, CS], dt.float32, tag=f"po{h}",
                               name=f"po{h}_{n}") for h in range(2)]
                q2 = q2p.tile([126, NG, CS], dt.bfloat16, tag="q2", name=f"q2{n}")
                nc.vector.tensor_mul(q2[:], q1[:], repa[:])
                for g in range(NG):
                    c0, ncg, P = _grp(g)
                    for h in range(2):
                        nc.tensor.matmul(
                            out=po[h][:], lhsT=wc9[0:P, g, h * C_:(h + 1) * C_],
                            rhs=q2[0:P, g, :], start=(g == 0), stop=(g == NG - 1))
                # finals: affines on ACT/DVE, residual adds on Pool
                osub = n % (OSC_ROWS // CH_ROWS)
                osl = slice(osub * CS, (osub + 1) * CS)
                xsl = slice(n * CS, (n + 1) * CS)
                ob = obt
                t0 = actp.tile([C_, CS], dt.bfloat16, tag="t0", name=f"t0_{n}")
                nc.scalar.activation(out=t0[:], in_=po[0][:], func=AF.Relu,
                                     scale=a2c2[:, 0, 0:1], bias=a2c2[:, 0, 1:2])
                nc.gpsimd.tensor_add(ob[0][:, osl], t0[:],
                                     x2[0][:, xsl].bitcast(dt.float32))
                t1 = actp.tile([C_, CS], dt.bfloat16, tag="t1", name=f"t1_{n}")
                with nc.allow_low_precision(reason="bn affine bf16 ok"):
                    nc.vector.tensor_scalar(
                        out=t1[:], in0=po[1][:], scalar1=a2c2[:, 1, 0:1],
                        scalar2=a2c2[:, 1, 1:2], op0=ALU.mult, op1=ALU.add)
                t1r = actp.tile([C_, CS], dt.bfloat16, tag="t1r", name=f"t1r_{n}")
                nc.vector.tensor_scalar(
                    out=t1r[:], in0=t1[:], scalar1=0.0, scalar2=None,
                    op0=ALU.max)
                nc.gpsimd.tensor_add(ob[1][:, osl], t1r[:],
                                     x2[1][:, xsl].bitcast(dt.float32))
                return ob

            def emit_store(n, ob):
                osc = n // (OSC_ROWS // CH_ROWS)
                for h in range(2):
                    nc.sync.dma_start(
                        out=out_d[h * C_:(h + 1) * C_,
                                  osc * OSC_ROWS * W:(osc + 1) * OSC_ROWS * W],
                        in_=ob[h][:])

            # ---- phase A interleaved with strip production ----
            # quarter q covers shifted-frame rows 20q..20q+19 (src rows up to
            # 20q+21 -> cv1 chunks through ceil((20q+21-1)/5)=4q+4)
            sts = {}
            warm = []
            for ch in range(NCH):
                emit_cv1(ch)
                if ch % 4 == 0 and ch // 4 < 3:
                    emit_x_piece(ch // 4 + 1)
                if ch == 4:
                    emit_hp9d_writes(0)
                    sts[0] = emit_strip_reads(0)
                    sts[1] = emit_strip_reads(1)
                elif ch == 9:
                    emit_hp9d_writes(1)
                elif ch == 11:
                    warm.append((0, emit_zde(0, sts[0])))
                elif ch == 13:
                    warm.append((1, emit_zde(1, sts[0])))
                elif ch == 14:
                    emit_hp9d_writes(2)
                elif ch == 15:
                    emit_hp9d_writes(3)

            # ---- main loop, software-pipelined by one chunk ----
            OS_SUBS = OSC_ROWS // CH_ROWS
            from collections import deque
            pend = deque(warm)
            DEPTH = 2

            def drain_one():
                pn, pq1 = pend.popleft()
                ob = emit_out(pn, pq1)
                if pn % OS_SUBS == OS_SUBS - 1:
                    emit_store(pn, ob)

            for rsc in range(NRSC):
                st = sts.pop(rsc) if rsc in sts else emit_strip_reads(rsc)
                for sub in range(RSUBS):
                    n = rsc * RSUBS + sub
                    if n < len(warm):
                        continue
                    q1 = emit_zde(n, st)
                    pend.append((n, q1))
                    if len(pend) > DEPTH:
                        drain_one()
            while pend:
                drain_one()

    _split_excess_waits(nc)
    return nc


def _split_excess_waits(nc):
    """This walrus build rejects >1 sync-wait on TPB_CTRL instructions and
    >2 elsewhere; redistribute onto same-engine wait-nops inserted before."""
    import concourse.mybir as mybir
    cnt = [0]
    for bb in nc.main_func.blocks:
        new_list = []
        changed = False
        for ins in bb.instructions:
            si = ins.sync_info
            lim = 1
            if si is not None and si.on_wait is not None and len(si.on_wait) > lim:
                waits = list(si.on_wait)
                head, tail = waits[:-lim], waits[-lim:]
                for w in head:
                    nop = mybir.InstNoOp(name=f"waitsplit-{cnt[0]}", ins=[], outs=[])
                    cnt[0] += 1
                    nop.engine = ins.engine
                    nop.sync_info = mybir.SyncInfo(on_wait=[w], on_update=[])
                    nop.bass_nofuse = True
                    try:
                        nc.register_instruction(nop)
                    except Exception:
                        pass
                    new_list.append(nop)
                ins.sync_info = mybir.SyncInfo(
                    on_wait=tail, on_update=list(si.on_update or []))
                changed = True
            new_list.append(ins)
        if changed:
            bb.instructions[:] = new_list


def _get_program():
    global _PROGRAM
    if _PROGRAM is None:
        _PROGRAM = _build_program()
    return _PROGRAM


def kernel(**inputs):
    from concourse.bass_utils import run_bass_kernel_spmd

    x = np.asarray(inputs['x'], dtype=np.float32)
    cst = _fold_constants(**{k: np.asarray(v, dtype=np.float32)
                             for k, v in inputs.items() if k != 'x'})
    nc = _get_program()
    base = {
        'w1t': cst['w1t'], 'a1c1': cst['a1c1'], 'zb': cst['zb'],
        'ones32': cst['ones32'], 'cg9': cst['cg9'],
        'wc9': cst['wc9'], 'a2c2': cst['a2c2'],
    }
    in_maps = [dict(base, xs=np.ascontiguousarray(x[b].reshape(C1, S)))
               for b in range(B)]
    res = run_bass_kernel_spmd(nc, in_maps, list(range(B)))
    out = np.stack([res.results[b]['out'].reshape(C2, H, W) for b in range(B)])
    return out.astype(np.float32)


# revision 6
# speedup vs baseline: 1.0340x; 1.0165x over previous
"""Trainium2 Bass kernel for nn_Bottleneck_11416023073044 (RFAConv bottleneck).

Data-parallel: 1 sample per NeuronCore (8 cores). cl-major interleave:
partition = cl*9 + idx (idx = patch index i for inputs / n for outputs)
within each 14-channel group (last group has 2).

Per-core pipeline per chunk (CS=400 = 5 rows x 80):
  cv1:    hp = silu(a1*(W1 @ x) + c1)   ACT sigmoid + DVE affine + DVE mul,
          interleaved with x piece loads and hp9d quarter writes
  strips: hp -> 9 shifted DRAM copies (quarter-frame windows) ->
          [126,10,820] read-sc tiles (10 rows, A/B buffers)
  z:      pz = zb_g^T @ strips_g                (bf16 matmul)
  e:      e9all[:,g,:] = exp(pz + cg9)          ACT -> bf16
  D:      pd[32m:+32, bank] = ones32^T @ e9     4 groups per PSUM bank at
                                                32-aligned tile_position
  recip:  rd = 1/pd                             one batched DVE reciprocal
  rep:    repall[:,g,:] <- rd slice             replicating DMA (stride-0
                                                inner dim), split SP/SWDGE
  q:      q1all = strips*e9all; q2all = q1all*repall   one DVE tt op each
          over all 10 groups ([126, 4000] per op, 2x_1p mode)
  out:    po[h] += wc9_g^T @ q2all[:,g,:]       bf16 matmuls, 10 groups
  final:  out = x + relu(a2*po + c2)            ACT/DVE affines + Pool adds
Software-pipelined two chunks deep; chunks 0-1 are warmed during phase A.
"""
import numpy as np
import ml_dtypes

EPS = 1e-5
B, C1, C2, H, W = 8, 256, 256, 80, 80
C_ = C2 // 2          # 128
NG = 10               # channel groups
GC = 14               # channels per group (last group has 2)
HP = H + 2            # 82
S = H * W             # 6400
CH_ROWS = 5           # rows per compute chunk
CS = CH_ROWS * W      # 400 columns per chunk
NCH = H // CH_ROWS    # 16 chunks
RSC_ROWS = 10         # rows per strip read-superchunk
NRSC = H // RSC_ROWS  # 8
RSUBS = RSC_ROWS // CH_ROWS  # 4 chunks per read-sc
OSC_ROWS = 20         # rows per output store block
FW = H * HP           # 6560 flat window length per shifted copy
HPFLAT = HP * HP + 2  # 6726


def _grp(g):
    c0 = g * GC
    ncg = min(GC, C_ - c0)
    return c0, ncg, 9 * ncg


def _fold_constants(W1, g1, b1, m1, v1, Wg, bg, gg, bgw, mg, vg, Wc, bc, g2, b2,
                    m2, v2):
    """Fold BN affines; build cl-major interleaved stationaries."""
    f32 = np.float32
    bf16 = ml_dtypes.bfloat16
    cst = {}
    a1 = (g1 / np.sqrt(v1 + EPS)).astype(f32)
    c1 = (b1 - m1 * a1).astype(f32)
    cst['a1c1'] = np.stack([a1, c1], axis=1)                  # [128, 2] f32

    cst['w1t'] = np.ascontiguousarray(W1.T).astype(f32)       # [256, 128] f32

    ag = gg / np.sqrt(vg + EPS)                               # [128, 9]
    A = (ag[:, :, None] * Wg).astype(f32)                     # [c, n, i]
    cg = (ag * (bg - mg) + bgw).astype(f32)                   # [128, 9]

    zb = np.zeros((126, NG, 126), f32)
    ones32 = np.zeros((126, 2, 32), f32)   # variant 0: ncg=14, 1: ncg=2
    cg9 = np.zeros((126, NG), f32)
    wc9 = np.zeros((126, NG, C2), f32)
    for g in range(NG):
        c0, ncg, P = _grp(g)
        for cl in range(ncg):
            c = c0 + cl
            for n in range(9):
                m = cl * 9 + n
                cg9[m, g] = cg[c, n]
                wc9[m, g, :] = Wc[:, c, n]
                for i in range(9):
                    zb[cl * 9 + i, g, m] = A[c, n, i]
    for v, ncg in ((0, GC), (1, 2)):
        for cl in range(ncg):
            for n in range(9):
                for j in range(32):
                    if j % ncg == cl:
                        ones32[cl * 9 + n, v, j] = 1.0
    cst['zb'] = zb.astype(bf16)
    cst['ones32'] = ones32.astype(bf16)
    cst['cg9'] = cg9
    cst['wc9'] = wc9.astype(bf16)

    a2 = (g2 / np.sqrt(v2 + EPS)).astype(f32)
    c2 = (b2 + a2 * (bc - m2)).astype(f32)
    a2c2 = np.zeros((C_, 2, 2), f32)
    for h in range(2):
        a2c2[:, h, 0] = a2[h * C_:(h + 1) * C_]
        a2c2[:, h, 1] = c2[h * C_:(h + 1) * C_]
    cst['a2c2'] = a2c2
    return cst


_PROGRAM = None


def _build_program():
    import concourse.bass as bass
    import concourse.tile as tile
    from concourse import mybir

    dt = mybir.dt
    AF = mybir.ActivationFunctionType
    ALU = mybir.AluOpType

    nc = bass.Bass("TRN2", target_bir_lowering=False, debug=False)

    xs_d = nc.dram_tensor("xs", [C1, S], dt.float32r, kind="ExternalInput")
    w1t_d = nc.dram_tensor("w1t", [C1, C_], dt.float32r, kind="ExternalInput")
    a1c1_d = nc.dram_tensor("a1c1", [C_, 2], dt.float32, kind="ExternalInput")
    zb_d = nc.dram_tensor("zb", [126, NG, 126], dt.bfloat16, kind="ExternalInput")
    ones_d = nc.dram_tensor("ones32", [126, 2, 32], dt.bfloat16, kind="ExternalInput")
    cg9_d = nc.dram_tensor("cg9", [126, NG], dt.float32, kind="ExternalInput")
    wc9_d = nc.dram_tensor("wc9", [126, NG, C2], dt.bfloat16, kind="ExternalInput")
    a2c2_d = nc.dram_tensor("a2c2", [C_, 2, 2], dt.float32, kind="ExternalInput")
    out_d = nc.dram_tensor("out", [C2, S], dt.float32, kind="ExternalOutput")
    hp9d = nc.dram_tensor("hp9d", [9, C_, FW], dt.bfloat16)

    with tile.TileContext(nc) as tc:
        with tc.tile_pool(name="singles", bufs=1) as singles, \
             tc.tile_pool(name="strips", bufs=1) as strips, \
             tc.tile_pool(name="q1p", bufs=3) as q1p, \
             tc.tile_pool(name="q2p", bufs=1) as q2p, \
             tc.tile_pool(name="rdp", bufs=2) as rdp, \
             tc.tile_pool(name="actp", bufs=2) as actp, \
             tc.tile_pool(name="obp", bufs=1) as obp, \
             tc.tile_pool(name="psz", bufs=3, space="PSUM") as psz, \
             tc.tile_pool(name="psd", bufs=1, space="PSUM") as psd, \
             tc.tile_pool(name="pso", bufs=1, space="PSUM") as pso:

            # ---- resident tiles + constant loads ----
            x2 = [singles.tile([C_, S], dt.float32r, tag=f"x{k}", name=f"x{k}")
                  for k in range(2)]
            w1t = [singles.tile([C_, C_], dt.float32r, tag=f"w1t{k}", name=f"w1t{k}")
                   for k in range(2)]
            for k in range(2):
                nc.sync.dma_start(out=w1t[k][:], in_=w1t_d[k * C_:(k + 1) * C_, :])
            a1c1 = singles.tile([C_, 2], dt.float32, tag="a1c1", name="a1c1")
            nc.sync.dma_start(out=a1c1[:], in_=a1c1_d[:])
            def emit_x_piece(piece):
                sl = slice(piece * (S // 4), (piece + 1) * (S // 4))
                for k in range(2):
                    nc.sync.dma_start(out=x2[k][:, sl],
                                      in_=xs_d[k * C_:(k + 1) * C_, sl])
            for k in range(2):
                nc.sync.dma_start(out=x2[k][:, 0:S // 8],
                                  in_=xs_d[k * C_:(k + 1) * C_, 0:S // 8])
            for k in range(2):
                nc.sync.dma_start(out=x2[k][:, S // 8:S // 4],
                                  in_=xs_d[k * C_:(k + 1) * C_, S // 8:S // 4])
            zb = singles.tile([126, NG, 126], dt.bfloat16, tag="zb", name="zb")
            nc.sync.dma_start(out=zb[:], in_=zb_d[:])
            ones32 = singles.tile([126, 2, 32], dt.bfloat16, tag="ones32", name="ones32")
            nc.sync.dma_start(out=ones32[:], in_=ones_d[:])
            cg9 = singles.tile([126, NG], dt.float32, tag="cg9", name="cg9")
            nc.sync.dma_start(out=cg9[:], in_=cg9_d[:])
            wc9 = singles.tile([126, NG, C2], dt.bfloat16, tag="wc9", name="wc9")
            nc.sync.dma_start(out=wc9[:], in_=wc9_d[:])
            a2c2 = singles.tile([C_, 2, 2], dt.float32, tag="a2c2", name="a2c2")
            nc.sync.dma_start(out=a2c2[:], in_=a2c2_d[:])

            hpfl = singles.tile([C_, HPFLAT], dt.bfloat16, tag="hp", name="hp")
            # zero only the pad border: top row, bottom row (+2 tail), side cols
            nc.vector.memset(hpfl[:, 0:HP], 0.0)
            nc.vector.memset(hpfl[:, (HP - 1) * HP:HPFLAT], 0.0)
            sideap = bass.AP(tensor=hpfl[:].tensor, offset=hpfl[:].offset + HP,
                             ap=[[HPFLAT, C_], [HP, H], [HP - 1, 2]])
            nc.vector.memset(sideap, 0.0)

            # big per-chunk group-batched tiles (explicit A/B by chunk parity)
            NPAR = 3
            e9all = [singles.tile([126, NG, CS], dt.bfloat16, tag=f"e9all{p}",
                                  name=f"e9all{p}") for p in range(NPAR)]
            repall = [singles.tile([126, NG, CS], dt.bfloat16, tag=f"repall{p}",
                                   name=f"repall{p}") for p in range(NPAR)]
            # one-time zero of group-9 stale partitions (never written later)
            for p in range(NPAR):
                nc.vector.memset(e9all[p][:, NG - 1, :], 0.0)
                nc.vector.memset(repall[p][:, NG - 1, :], 0.0)

            hp3 = hpfl[:, 0:HP * HP].rearrange("p (a b) -> p a b", a=HP)

            # ---- phase A: cv1 into padded frame ----
            def emit_cv1(ch):
                y0 = ch * CH_ROWS
                ph = psz.tile([C_, CS], dt.float32, tag="pz", name="ph")
                for k in range(2):
                    nc.tensor.matmul(
                        out=ph[:], lhsT=w1t[k][:],
                        rhs=x2[k][:, y0 * W:(y0 + CH_ROWS) * W],
                        start=(k == 0), stop=(k == 1))
                yb = actp.tile([C_, CS], dt.bfloat16, tag="yb", name="yb")
                nc.vector.tensor_scalar(
                    out=yb[:], in0=ph[:], scalar1=a1c1[:, 0:1],
                    scalar2=a1c1[:, 1:2], op0=ALU.mult, op1=ALU.add)
                sg = actp.tile([C_, CS], dt.bfloat16, tag="sg", name="sg")
                nc.scalar.activation(out=sg[:], in_=ph[:], func=AF.Sigmoid,
                                     scale=a1c1[:, 0:1], bias=a1c1[:, 1:2])
                nc.vector.tensor_mul(hp3[:, 1 + y0:1 + y0 + CH_ROWS, 1:1 + W],
                                     yb[:], sg[:])

            def emit_hp9d_writes(q):
                f0 = q * (FW // 4)
                for i in range(9):
                    d = (i // 3) * HP + (i % 3)
                    nc.sync.dma_start(
                        out=hp9d[i, :, f0:f0 + FW // 4],
                        in_=hpfl[:, d + f0:d + f0 + FW // 4])

            # ---- persistent PSUM D banks (one 3-bank tile) ----
            pd = psd.tile([C_, 3, 512], dt.float32, tag="pd", name="pd")
            nc.vector.memset(pd[:], 1.0)

            stp = [strips.tile([126, NG, RSC_ROWS * HP], dt.bfloat16,
                               tag=f"st{p}", name=f"st{p}") for p in range(2)]
            for p in range(2):
                nc.vector.memset(stp[p][:, NG - 1, :], 0.0)
            obt = [obp.tile([C_, OSC_ROWS * W], dt.float32, tag=f"ob{h}",
                            name=f"ob{h}") for h in range(2)]

            def emit_strip_reads(rsc):
                ys = rsc * RSC_ROWS
                st = stp[rsc % 2]
                for g in range(NG):
                    c0, ncg, P = _grp(g)
                    srcap = bass.AP(
                        tensor=hp9d[0].tensor, offset=c0 * FW + ys * HP,
                        ap=[[FW, ncg], [C_ * FW, 9], [1, RSC_ROWS * HP]])
                    eng = nc.gpsimd if g % 2 == 0 else nc.sync
                    eng.dma_start(out=st[0:P, g, :], in_=srcap)
                return st

            def emit_d(g, e9a):
                c0, ncg, P = _grp(g)
                v = 0 if ncg == GC else 1
                b, m = g // 4, g % 4
                nc.tensor.matmul(out=pd[32 * m:32 * m + 32, b, 0:CS],
                                 lhsT=ones32[0:P, v, :], rhs=e9a[0:P, g, :],
                                 start=True, stop=True, tile_position=(0, 32 * m),
                                 skip_group_check=True)

            def emit_recip_reps(n, rd, repa):
                with nc.allow_low_precision(reason="softmax recip bf16"):
                    nc.vector.reciprocal(
                        out=rd[:].rearrange("p (b e) -> p b e", b=3),
                        in_=pd[:, :, 0:CS])
                for g in range(NG):
                    c0, ncg, P = _grp(g)
                    b, m = g // 4, g % 4
                    srcap = bass.AP(
                        tensor=rd[:].tensor,
                        offset=rd[:].offset + (32 * m) * (3 * CS) + b * CS,
                        ap=[[3 * CS, ncg], [0, 9], [1, CS]])
                    eng = nc.gpsimd if g % 2 == 0 else nc.sync
                    eng.dma_start(out=repa[0:P, g, :], in_=srcap)

            def emit_zde(n, st):
                """z, exp, D, recip, rep, q1 for chunk n."""
                sub = n % RSUBS
                par = n % NPAR
                e9a, repa = e9all[par], repall[par]
                str3 = st[:].rearrange("p g (r c) -> p g r c", c=HP)
                rd = rdp.tile([C_, 3 * CS], dt.bfloat16, tag="rd", name=f"rd{n}")

                for g in range(NG):
                    c0, ncg, P = _grp(g)
                    rhs = str3[0:P, g, sub * CH_ROWS:(sub + 1) * CH_ROWS, 0:W]
                    pz = psz.tile([126, CS], dt.float32, tag="pz",
                                  name=f"pz{n}_{g}")
                    nc.tensor.matmul(out=pz[0:P, :], lhsT=zb[0:P, g, :][:, 0:P],
                                     rhs=rhs, start=True, stop=True)
                    nc.scalar.activation(out=e9a[0:P, g, :], in_=pz[0:P, :],
                                         func=AF.Exp, bias=cg9[0:P, g:g + 1])
                    if g > 0:
                        emit_d(g - 1, e9a)
                emit_d(NG - 1, e9a)
                emit_recip_reps(n, rd, repa)
                # one batched q1 over all groups
                q1 = q1p.tile([126, NG, CS], dt.bfloat16, tag="q1", name=f"q1{n}")
                sview = str3[0:126, :, sub * CH_ROWS:(sub + 1) * CH_ROWS, 0:W]
                nc.vector.tensor_mul(q1[:], sview, e9a[:])
                return q1

            def emit_out(n, q1):
                """q2 mul + out matmuls + finals for chunk n."""
                par = n % NPAR
                repa = repall[par]
                po = [pso.tile([C_, CS], dt.float32, tag=f"po{h}",
                               name=f"po{h}_{n}") for h in range(2)]
                q2 = q2p.tile([126, NG, CS], dt.bfloat16, tag="q2", name=f"q2{n}")
                nc.vector.tensor_mul(q2[:], q1[:], repa[:])
                for g in range(NG):
                    c0, ncg, P = _grp(g)
                    for h in range(2):
                        nc.tensor.matmul(
                            out=po[h][:], lhsT=wc9[0:P, g, h * C_:(h + 1) * C_],
                            rhs=q2[0:P, g, :], start=(g == 0), stop=(g == NG - 1))
                # finals: affines on ACT/DVE, residual adds on Pool
                osub = n % (OSC_ROWS // CH_ROWS)
                osl = slice(osub * CS, (osub + 1) * CS)
                xsl = slice(n * CS, (n + 1) * CS)
                ob = obt
                t0 = actp.tile([C_, CS], dt.bfloat16, tag="t0", name=f"t0_{n}")
                nc.scalar.activation(out=t0[:], in_=po[0][:], func=AF.Relu,
                                     scale=a2c2[:, 0, 0:1], bias=a2c2[:, 0, 1:2])
                nc.gpsimd.tensor_add(ob[0][:, osl], t0[:],
                                     x2[0][:, xsl].bitcast(dt.float32))
                t1 = actp.tile([C_, CS], dt.bfloat16, tag="t1", name=f"t1_{n}")
                with nc.allow_low_precision(reason="bn affine bf16 ok"):
                    nc.vector.tensor_scalar(
                        out=t1[:], in0=po[1][:], scalar1=a2c2[:, 1, 0:1],
                        scalar2=a2c2[:, 1, 1:2], op0=ALU.mult, op1=ALU.add)
                t1r = actp.tile([C_, CS], dt.bfloat16, tag="t1r", name=f"t1r_{n}")
                nc.vector.tensor_scalar(
                    out=t1r[:], in0=t1[:], scalar1=0.0, scalar2=None,
                    op0=ALU.max)
                nc.gpsimd.tensor_add(ob[1][:, osl], t1r[:],
                                     x2[1][:, xsl].bitcast(dt.float32))
                return ob

            def emit_store(n, ob, half):
                osc = n // (OSC_ROWS // CH_ROWS)
                HB = OSC_ROWS * W // 2
                o0 = osc * OSC_ROWS * W + half * HB
                for h in range(2):
                    nc.sync.dma_start(
                        out=out_d[h * C_:(h + 1) * C_, o0:o0 + HB],
                        in_=ob[h][:, half * HB:(half + 1) * HB])

            # ---- phase A interleaved with strip production ----
            # quarter q covers shifted-frame rows 20q..20q+19 (src rows up to
            # 20q+21 -> cv1 chunks through ceil((20q+21-1)/5)=4q+4)
            sts = {}
            warm = []
            for ch in range(NCH):
                emit_cv1(ch)
                if ch % 4 == 0 and ch // 4 < 3:
                    emit_x_piece(ch // 4 + 1)
                if ch == 4:
                    emit_hp9d_writes(0)
                    sts[0] = emit_strip_reads(0)
                    sts[1] = emit_strip_reads(1)
                elif ch == 9:
                    emit_hp9d_writes(1)
                elif ch == 11:
                    warm.append((0, emit_zde(0, sts[0])))
                elif ch == 13:
                    warm.append((1, emit_zde(1, sts[0])))
                elif ch == 14:
                    emit_hp9d_writes(2)
                elif ch == 15:
                    emit_hp9d_writes(3)

            # ---- main loop, software-pipelined by one chunk ----
            OS_SUBS = OSC_ROWS // CH_ROWS
            from collections import deque
            pend = deque(warm)
            DEPTH = 2

            def drain_one():
                pn, pq1 = pend.popleft()
                ob = emit_out(pn, pq1)
                if pn % OS_SUBS == OS_SUBS // 2 - 1:
                    emit_store(pn, ob, 0)
                elif pn % OS_SUBS == OS_SUBS - 1:
                    emit_store(pn, ob, 1)

            for rsc in range(NRSC):
                st = sts.pop(rsc) if rsc in sts else emit_strip_reads(rsc)
                for sub in range(RSUBS):
                    n = rsc * RSUBS + sub
                    if n < len(warm):
                        continue
                    q1 = emit_zde(n, st)
                    pend.append((n, q1))
                    if len(pend) > DEPTH:
                        drain_one()
            while pend:
                drain_one()

    _split_excess_waits(nc)
    return nc


def _split_excess_waits(nc):
    """This walrus build rejects >1 sync-wait on TPB_CTRL instructions and
    >2 elsewhere; redistribute onto same-engine wait-nops inserted before."""
    import concourse.mybir as mybir
    cnt = [0]
    for bb in nc.main_func.blocks:
        new_list = []
        changed = False
        for ins in bb.instructions:
            si = ins.sync_info
            lim = 1
            if si is not None and si.on_wait is not None and len(si.on_wait) > lim:
                waits = list(si.on_wait)
                head, tail = waits[:-lim], waits[-lim:]
                for w in head:
                    nop = mybir.InstNoOp(name=f"waitsplit-{cnt[0]}", ins=[], outs=[])
                    cnt[0] += 1
                    nop.engine = ins.engine
                    nop.sync_info = mybir.SyncInfo(on_wait=[w], on_update=[])
                    nop.bass_nofuse = True
                    try:
                        nc.register_instruction(nop)
                    except Exception:
                        pass
                    new_list.append(nop)
                ins.sync_info = mybir.SyncInfo(
                    on_wait=tail, on_update=list(si.on_update or []))
                changed = True
            new_list.append(ins)
        if changed:
            bb.instructions[:] = new_list


def _get_program():
    global _PROGRAM
    if _PROGRAM is None:
        _PROGRAM = _build_program()
    return _PROGRAM


def kernel(**inputs):
    from concourse.bass_utils import run_bass_kernel_spmd

    x = np.asarray(inputs['x'], dtype=np.float32)
    cst = _fold_constants(**{k: np.asarray(v, dtype=np.float32)
                             for k, v in inputs.items() if k != 'x'})
    nc = _get_program()
    base = {
        'w1t': cst['w1t'], 'a1c1': cst['a1c1'], 'zb': cst['zb'],
        'ones32': cst['ones32'], 'cg9': cst['cg9'],
        'wc9': cst['wc9'], 'a2c2': cst['a2c2'],
    }
    in_maps = [dict(base, xs=np.ascontiguousarray(x[b].reshape(C1, S)))
               for b in range(B)]
    res = run_bass_kernel_spmd(nc, in_maps, list(range(B)))
    out = np.stack([res.results[b]['out'].reshape(C2, H, W) for b in range(B)])
    return out.astype(np.float32)
